# revision 20
# baseline (speedup 1.0000x reference)
"""Multi-head attention (B=4, S=2048, D=1024, H=16, Dh=64) on 8 trn2 cores.

Sharding: core c -> batch b=c//2, head-group g=c%2 (8 heads = 512 qkv cols).
Host folds 1/sqrt(Dh) into Wq/bq, drops bk (softmax-invariant), splits bo
across the two cores of each batch. Each core computes a transposed partial
output outT [1024, 2048]; host sums core pairs and transposes.

v4: scalar-exp-floor schedule. Scores for the A/B head pair land in one
persistent 4-bank PSUM tile st2 = [Aqq0|Aqq1|Bqq0|Bqq1]; exp calls are
split by query-half (qq), so each exp frees exactly the two banks the next
score pair writes (1-iteration score lookahead, bank-granular WAR). ctx
lags one iteration via a 6-deep p2 ring; B-head ctx replays per 4-kc group
as two 4-matmul bursts. The PE is kept dense (DVFS: idle slivers drop the
clock to 1.2GHz) by draining exactly two 2-matmul filler steps per
iteration (supply 256 steps = demand). x/weights are SBUF-resident with
single-instruction DMAs (issue cost ~650ns each on the Sync queue);
wo/cn are bf16; v-bias folds into the DVE eviction; softmax reciprocal is
broadcast via a K=1 PE matmul instead of gpsimd; the output projection for
the last query quarter pre-accumulates c2<3 partials so the tail only runs
one matmul per block after the final normalize.
"""
import numpy as np
import ml_dtypes
from collections import deque
from contextlib import ExitStack

import concourse.bass as bass
import concourse.bacc as bacc
import concourse.mybir as mybir
import concourse.tile as tile
from concourse.bass_utils import run_bass_kernel_spmd

F32 = mybir.dt.float32
BF16 = mybir.dt.bfloat16
NP_BF16 = ml_dtypes.bfloat16

B = 4
S = 2048
D = 1024
COLS = 512          # qkv cols per core (8 heads x 64)
NHEAD = 8           # heads per core
N = 512             # matmul moving free dim
DCH = D // 128      # 8 contraction chunks for projections
SC = S // N         # 4 seq chunks of 512
CC = COLS // 128    # 4 col chunks (head pairs)
KC = S // 128       # 16 key chunks
QH = 2              # query halves of 1024
QHW = S // QH       # 1024
NITER = QH * CC * KC
P2N = 6

_CACHE = {}


def _build():
    nc = bacc.Bacc("TRN2", target_bir_lowering=False, debug=False, num_devices=8)

    xt = nc.declare_dram_parameter("xt", [D, S], BF16, isOutput=False)
    wqt = nc.declare_dram_parameter("wqt", [D, COLS], BF16, isOutput=False)
    wkt = nc.declare_dram_parameter("wkt", [D, COLS], BF16, isOutput=False)
    wvt = nc.declare_dram_parameter("wvt", [D, COLS], BF16, isOutput=False)
    wot = nc.declare_dram_parameter("wot", [COLS, D], BF16, isOutput=False)
    bq = nc.declare_dram_parameter("bq", [128, CC], F32, isOutput=False)
    bv = nc.declare_dram_parameter("bv", [1, COLS], F32, isOutput=False)
    bo2 = nc.declare_dram_parameter("bo2", [128, DCH], F32, isOutput=False)
    out = nc.declare_dram_parameter("out", [D, S], BF16, isOutput=True)

    with ExitStack() as ctx:
        tc = ctx.enter_context(tile.TileContext(nc))

        # ---------------- persistent SBUF ----------------
        const = ctx.enter_context(tc.tile_pool(name="const", bufs=1))
        ones_f32 = const.tile([128, 128], F32, tag="ones_f32")
        nc.vector.memset(ones_f32[:], 1.0)
        bq_t = const.tile([128, CC], F32, tag="bq")
        nc.sync.dma_start(out=bq_t[:], in_=bq[:])
        bo_t = const.tile([128, DCH], F32, tag="bo")
        nc.sync.dma_start(out=bo_t[:], in_=bo2[:])
        bv_row = const.tile([1, COLS], F32, tag="bv_row")
        nc.sync.dma_start(out=bv_row[:], in_=bv[:])
        bv_bc = const.tile([128, COLS], F32, tag="bv_bc")
        nc.gpsimd.partition_broadcast(bv_bc[:], bv_row[:])

        # weights resident: one consolidated tile per matrix -> 1-instr DMAs
        wpool = ctx.enter_context(tc.tile_pool(name="w", bufs=1))
        wqall = wpool.tile([128, DCH * COLS], BF16, tag="wq", name="wq")
        wkall = wpool.tile([128, DCH * COLS], BF16, tag="wk", name="wk")
        wvall = wpool.tile([128, DCH * COLS], BF16, tag="wv", name="wv")
        woall = wpool.tile([128, CC * D], BF16, tag="wo", name="wo")

        def wsl(t, d, lo, hi):
            return t[:, d * COLS + lo:d * COLS + hi]

        # activations
        qkv = ctx.enter_context(tc.tile_pool(name="qkv", bufs=1))
        qT = [qkv.tile([128, S], BF16, tag=f"qt{c}", name=f"qt{c}") for c in range(CC)]
        kT = [qkv.tile([128, S], BF16, tag=f"kt{c}", name=f"kt{c}") for c in range(CC)]
        v_sb = [qkv.tile([128, NHEAD * 65], BF16, tag=f"v{i}", name=f"v{i}")
                for i in range(KC)]
        cn = [qkv.tile([128, S], BF16, tag=f"cn{c}", name=f"cn{c}") for c in range(CC)]
        xall = qkv.tile([128, DCH * S], BF16, tag="xall", name="xall")
        opart = [qkv.tile([128, N], BF16, tag=f"op{i}", name=f"op{i}")
                 for i in range(16)]

        for i in range(KC):
            va = v_sb[i][:].rearrange("p (h c) -> p h c", c=65)
            nc.vector.tensor_copy(
                out=va[:, :, 64:65],
                in_=ones_f32[:, 0:NHEAD].rearrange("p (h c) -> p h c", c=1),
            )

        # p2 ring: [A-qq0 | A-qq1 | B-qq0 | B-qq1] each 512 wide
        ppool = ctx.enter_context(tc.tile_pool(name="p", bufs=1))
        p2 = [ppool.tile([128, 2 * QHW], BF16, tag=f"p2_{i}", name=f"p2_{i}")
              for i in range(P2N)]

        cpool = ctx.enter_context(tc.tile_pool(name="cx", bufs=1))
        caccA = [cpool.tile([65, QHW], F32, tag=f"cxA{i}", name=f"cxA{i}")
                 for i in range(2)]
        caccB = [cpool.tile([65, QHW], F32, tag=f"cxB{i}", name=f"cxB{i}")
                 for i in range(2)]
        rpool = ctx.enter_context(tc.tile_pool(name="r", bufs=1))
        rbpool = ctx.enter_context(tc.tile_pool(name="rb", bufs=1))
        opool = ctx.enter_context(tc.tile_pool(name="osb", bufs=4))

        # PSUM: st2(4 banks) sh0(1) sh1(1) pj0(1) pj1(1) = 8 banks
        psum = ctx.enter_context(tc.tile_pool(name="ps", bufs=1, space="PSUM"))
        stq = [psum.tile([128, QHW], F32, tag=f"stq{q}", name=f"stq{q}")
               for q in range(2)]
        sh = [psum.tile([65, N], F32, tag=f"sh{q}", name=f"sh{q}") for q in range(2)]
        pj = [psum.tile([128, N], F32, tag=f"pj{q}", name=f"pj{q}") for q in range(2)]

        # ---------------- filler step queue ----------------
        fillers = deque()
        done = set()
        pjctr = [0]

        def dma_xsc(sc):
            def go():
                src = xt[:, sc * N:(sc + 1) * N].rearrange(
                    "(d p) s -> p d s", p=128)
                dst = xall[:].rearrange("p (d s) -> p d s", s=S)[
                    :, :, sc * N:(sc + 1) * N]
                nc.sync.dma_start(out=dst, in_=src)
            return go

        def dma_w_chunk(tdst, src, c):
            def go():
                s = src[:, c * 128:(c + 1) * 128].rearrange(
                    "(d p) s -> p d s", p=128)
                d = tdst[:].rearrange("p (d s) -> p d s", s=COLS)[
                    :, :, c * 128:(c + 1) * 128]
                nc.sync.dma_start(out=d, in_=s)
            return go

        def dma_w_full(tdst, src):
            def go():
                w = src.shape[1]
                nc.sync.dma_start(
                    out=tdst[:].rearrange("p (d s) -> p d s", s=w),
                    in_=src[:].rearrange("(d p) s -> p d s", p=128))
            return go

        def xsl(d, lo, hi):
            return xall[:, d * S + lo:d * S + hi]

        # filler steps: 2 matmuls each; 4 steps per projection unit
        def add_proj_qk(proj, c, sc, lo=0, hi=N, key=None, one_step=False):
            wsrc = wqall if proj == "q" else wkall
            dst = qT if proj == "q" else kT
            box = {}
            if key is None:
                key = (proj, c, sc)
            w = hi - lo

            def mk(d0, dn):
                def go():
                    if d0 == 0:
                        box["ps"] = pj[pjctr[0] % 2]
                        pjctr[0] += 1
                    ps = box["ps"]
                    for d in range(d0, dn):
                        nc.tensor.matmul(
                            ps[:, 0:w], wsl(wsrc, d, c * 128, (c + 1) * 128),
                            xsl(d, sc * N + lo, sc * N + hi),
                            start=(d == 0), stop=(d == DCH - 1),
                        )
                    if dn == DCH:
                        if proj == "q":
                            nc.vector.tensor_scalar_add(
                                out=dst[c][:, sc * N + lo:sc * N + hi],
                                in0=ps[:, 0:w], scalar1=bq_t[:, c:c + 1],
                            )
                        else:
                            nc.vector.tensor_copy(
                                out=dst[c][:, sc * N + lo:sc * N + hi],
                                in_=ps[:, 0:w],
                            )
                return go
            if one_step:
                fillers.append((key, True, mk(0, DCH), True))
            else:
                for d0 in range(0, DCH, 2):
                    fillers.append((key if d0 == DCH - 2 else None, True,
                                    mk(d0, d0 + 2), d0 == 0))

        def add_proj_v(kc):
            box = {}
            key = ("v", kc)

            def mk(d0):
                def go():
                    if d0 == 0:
                        box["ps"] = pj[pjctr[0] % 2]
                        pjctr[0] += 1
                    ps = box["ps"]
                    for d in (d0, d0 + 1):
                        nc.tensor.matmul(
                            ps[:], xsl(d, kc * 128, (kc + 1) * 128),
                            wsl(wvall, d, 0, COLS),
                            start=(d == 0), stop=(d == DCH - 1),
                        )
                    if d0 == DCH - 2:
                        dst = v_sb[kc][:].rearrange("p (h c) -> p h c", c=65)
                        src = ps[:].rearrange("p (h c) -> p h c", c=64)
                        bvr = bv_bc[:].rearrange("p (h c) -> p h c", c=64)
                        nc.vector.tensor_tensor(
                            out=dst[:, :, 0:64], in0=src[:], in1=bvr[:],
                            op=mybir.AluOpType.add,
                        )
                return go
            for d0 in range(0, DCH, 2):
                fillers.append((key if d0 == DCH - 2 else None, True, mk(d0),
                                d0 == 0))

        def add_out_proj(e, qc):
            """Full output projection (contract all 4 c2) -> DRAM."""
            box = {}
            key = ("o", e, qc)

            def mk(c0):
                def go():
                    if c0 == 0:
                        box["ps"] = pj[pjctr[0] % 2]
                        pjctr[0] += 1
                    ps = box["ps"]
                    for c2 in (c0, c0 + 1):
                        nc.tensor.matmul(
                            ps[:], woall[:, c2 * D + e * 128:c2 * D + (e + 1) * 128],
                            cn[c2][:, qc * N:(qc + 1) * N],
                            start=(c2 == 0), stop=(c2 == CC - 1),
                        )
                    if c0 == CC - 2:
                        o_t = opool.tile([128, N], BF16, tag="o", name=f"o{e}_{qc}")
                        nc.vector.tensor_scalar_add(
                            out=o_t[:], in0=ps[:], scalar1=bo_t[:, e:e + 1],
                        )
                        nc.sync.dma_start(
                            out=out[e * 128:(e + 1) * 128, qc * N:(qc + 1) * N],
                            in_=o_t[:],
                        )
                return go
            for c0 in range(0, CC, 2):
                fillers.append((key if c0 == CC - 2 else None, True, mk(c0),
                                c0 == 0))

        def add_out_part(e, qc):
            """Partial output projection: c2 in {0,1,2} + bias -> opart (bf16)."""
            box = {}
            key = ("pp", e, qc)

            def s1():
                box["ps"] = pj[pjctr[0] % 2]
                pjctr[0] += 1
                for c2 in (0, 1):
                    nc.tensor.matmul(
                        box["ps"][:], woall[:, c2 * D + e * 128:c2 * D + (e + 1) * 128],
                        cn[c2][:, qc * N:(qc + 1) * N],
                        start=(c2 == 0), stop=False,
                    )

            def s2():
                ps = box["ps"]
                nc.tensor.matmul(
                    ps[:], woall[:, 2 * D + e * 128:2 * D + (e + 1) * 128],
                    cn[2][:, qc * N:(qc + 1) * N],
                    start=False, stop=True,
                )
                nc.vector.tensor_scalar_add(
                    out=opart[e * 2 + (qc - 2)][:], in0=ps[:],
                    scalar1=bo_t[:, e:e + 1],
                )
            fillers.append((None, True, s1, True))
            fillers.append((key, True, s2, False))

        def pop_one():
            key, pe, fn, _ = fillers.popleft()
            fn()
            if key is not None:
                done.add(key)
            return pe

        def drain(n):
            emitted = 0
            while fillers and emitted < n:
                if pop_one():
                    emitted += 1

        def drain_boundary():
            """Pop until the queue front starts a fresh unit, so both pj
            banks are closed (no open PSUM accumulation group)."""
            while fillers and not fillers[0][3]:
                pop_one()

        def drain_until(key):
            while key not in done:
                assert fillers, f"filler queue empty but {key} needed"
                pop_one()

        # lead-in: minimal critical path first
        fillers.append((None, False, dma_w_chunk(wkall, wkt, 0), True))
        fillers.append((None, False, dma_xsc(0), True))
        fillers.append((None, False, dma_w_chunk(wqall, wqt, 0), True))
        add_proj_qk("k", 0, 0, lo=0, hi=128, key=("k00a",), one_step=True)
        add_proj_qk("q", 0, 0)
        fillers.append((None, False, dma_xsc(1), True))
        add_proj_qk("q", 0, 1)
        add_proj_qk("k", 0, 0, lo=128, hi=N)
        fillers.append((None, False, dma_w_full(wvall, wvt), True))
        fillers.append((None, False, dma_xsc(2), True))
        fillers.append((None, False, dma_xsc(3), True))
        for kc in range(4):
            add_proj_v(kc)
        add_proj_qk("k", 0, 1)
        for kc in range(4, 8):
            add_proj_v(kc)
        add_proj_qk("k", 0, 2)
        for kc in range(8, 12):
            add_proj_v(kc)
        add_proj_qk("k", 0, 3)
        for kc in range(12, 16):
            add_proj_v(kc)
        for cc_ in range(1, CC):
            fillers.append((None, False, dma_w_chunk(wkall, wkt, cc_), True))
            fillers.append((None, False, dma_w_chunk(wqall, wqt, cc_), True))
        fillers.append((None, False, dma_w_full(woall, wot), True))
        add_proj_qk("q", 0, 2)
        add_proj_qk("q", 0, 3)
        for cc_ in range(1, CC):
            add_proj_qk("k", cc_, 0)
            add_proj_qk("q", cc_, 0)
            add_proj_qk("q", cc_, 1)
            add_proj_qk("k", cc_, 1)
            add_proj_qk("k", cc_, 2)
            add_proj_qk("k", cc_, 3)
            add_proj_qk("q", cc_, 2)
            add_proj_qk("q", cc_, 3)

        # ---------------- main flat pipeline ----------------
        def qck(j):
            return j // (CC * KC), (j // KC) % CC, j % KC

        def emit_sp(j, qq):
            qh, c, kc = qck(j)
            if c == 0 and kc == 0:
                drain_until(("k00a",))
            else:
                drain_until(("k", c, kc // 4))
            drain_until(("q", c, 2 * qh))
            drain_until(("q", c, 2 * qh + 1))
            qs = qh * QHW + qq * N
            for po, boff in ((0, 0), (64, N)):
                nc.tensor.matmul(
                    stq[qq][:, boff:boff + N],
                    kT[c][po:po + 64, kc * 128:(kc + 1) * 128],
                    qT[c][po:po + 64, qs:qs + N],
                    start=True, stop=True,
                )

        def emit_exp(j, qq):
            nc.scalar.activation(
                p2[j % P2N][:, qq * QHW:(qq + 1) * QHW],
                stq[qq][:],
                mybir.ActivationFunctionType.Exp,
            )

        def emit_ctxA(j, qq):
            qh, c, kc = qck(j)
            drain_until(("v", kc))
            gi = kc % 4
            lvA = v_sb[kc][:, 2 * c * 65:(2 * c + 1) * 65]
            nc.tensor.matmul(
                sh[qq][:], lvA, p2[j % P2N][:, qq * QHW:qq * QHW + N],
                start=(gi == 0), stop=(gi == 3),
            )

        def emit_evict(stream, g, qq):
            acc = caccA if stream == "A" else caccB
            sl = slice(qq * N, (qq + 1) * N)
            if g == 0:
                nc.vector.tensor_copy(out=acc[0][:, sl], in_=sh[qq][:])
            else:
                nc.vector.tensor_tensor(
                    out=acc[g % 2][:, sl], in0=sh[qq][:],
                    in1=acc[(g + 1) % 2][:, sl], op=mybir.AluOpType.add,
                )

        def emit_burstB(j, qq):
            qh, c, kc = qck(j)
            g = kc // 4
            base = j - kc
            for kb in range(4 * g, 4 * g + 4):
                lvB = v_sb[kb][:, (2 * c + 1) * 65:(2 * c + 2) * 65]
                nc.tensor.matmul(
                    sh[qq][:], lvB,
                    p2[(base + kb) % P2N][:, qq * QHW + N:(qq + 1) * QHW],
                    start=(kb == 4 * g), stop=(kb == 4 * g + 3),
                )

        def emit_normalize(qh, c, stream):
            acc, po = (caccA, 0) if stream == "A" else (caccB, 64)
            cx = acc[(KC // 4 - 1) % 2]
            l_t = rpool.tile([1, QHW], F32, tag="l", name=f"l{stream}{qh}_{c}")
            nc.vector.tensor_copy(out=l_t[:], in_=cx[64:65, :])
            r_t = rpool.tile([1, QHW], F32, tag="r", name=f"r{stream}{qh}_{c}")
            nc.vector.reciprocal_approx_fast(r_t[:], l_t[:])
            rb_t = rbpool.tile([64, QHW], F32, tag="rb",
                               name=f"rb{stream}{qh}_{c}")
            nc.gpsimd.partition_broadcast(rb_t[:], r_t[:])
            nc.vector.tensor_tensor(
                out=cn[c][po:po + 64, qh * QHW:(qh + 1) * QHW],
                in0=cx[0:64, :], in1=rb_t[:],
                op=mybir.AluOpType.mult,
            )

        def emit_ctx_block(j):
            qh, c, kc = qck(j)
            emit_ctxA(j, 0)
            emit_ctxA(j, 1)
            if kc % 4 == 3:
                g = kc // 4
                emit_evict("A", g, 0)
                emit_evict("A", g, 1)

        def emit_burst_block(j):
            qh, c, kc = qck(j)
            g = kc // 4
            emit_burstB(j, 0)
            emit_evict("B", g, 0)
            emit_burstB(j, 1)
            emit_evict("B", g, 1)

        def emit_burst_half2(j):
            qh, c, kc = qck(j)
            g = kc // 4
            emit_burstB(j, 1)
            emit_evict("B", g, 1)

        emit_sp(0, 0)
        emit_exp(0, 0)
        emit_sp(0, 1)
        emit_exp(0, 1)

        for it in range(NITER):
            j_next = it + 1
            j_ctx = it - 1
            burst = j_ctx >= 0 and (j_ctx % KC) % 4 == 3
            if j_next < NITER:
                emit_sp(j_next, 0)
            if j_ctx >= 0:
                emit_ctx_block(j_ctx)
            if j_next < NITER:
                emit_exp(j_next, 0)
            j_norm = it - 4
            if j_norm >= 0 and j_norm % KC == KC - 1:
                qhn, cn_, _ = qck(j_norm)
                emit_normalize(qhn, cn_, "A")
                emit_normalize(qhn, cn_, "B")
            dn = 2 if (it % KC) >= KC - 4 else 1
            drain(dn)
            if j_next < NITER:
                emit_sp(j_next, 1)
                emit_exp(j_next, 1)
            if burst:
                emit_burst_block(j_ctx)
            else:
                drain(dn)
            if it == 72:
                for e in range(DCH):
                    for qc in range(2):
                        add_out_proj(e, qc)
            if it == 116:
                for e in range(DCH):
                    for qc in range(2, SC):
                        add_out_part(e, qc)

        # epilogue
        emit_ctx_block(NITER - 1)
        emit_burst_block(NITER - 1)
        emit_normalize(QH - 1, CC - 1, "A")
        emit_normalize(QH - 1, CC - 1, "B")
        while fillers:
            pop_one()
        # tail: finish qc2/3 output blocks: 1 matmul (c2=3) + add partial.
        # Rotate across 4 free PSUM tiles (stq0/stq1/pj0/pj1) for a deep
        # MM->DVE->DMA pipeline.
        slots = [stq[0], stq[1], pj[0], pj[1]]
        ti = 0
        for e in range(DCH):
            for qc in range(2, SC):
                ps = slots[ti % 4]
                ti += 1
                nc.tensor.matmul(
                    ps[:, 0:N], woall[:, 3 * D + e * 128:3 * D + (e + 1) * 128],
                    cn[3][:, qc * N:(qc + 1) * N], start=True, stop=True,
                )
                o_t = opool.tile([128, N], BF16, tag="o", name=f"ot{e}_{qc}")
                nc.vector.tensor_tensor(
                    out=o_t[:], in0=ps[:, 0:N], in1=opart[e * 2 + (qc - 2)][:],
                    op=mybir.AluOpType.add,
                )
                nc.sync.dma_start(
                    out=out[e * 128:(e + 1) * 128, qc * N:(qc + 1) * N],
                    in_=o_t[:],
                )

    nc.compile()
    return nc


def _get_nc():
    if "nc" not in _CACHE:
        _CACHE["nc"] = _build()
    return _CACHE["nc"]


def _in_maps(x, Wq, bq, Wk, Wv, bv, Wo, bo):
    maps = []
    for core in range(8):
        b, g = core // 2, core % 2
        cols = slice(g * COLS, (g + 1) * COLS)
        maps.append({
            "xt": np.ascontiguousarray(x[b].T).astype(NP_BF16),
            "wqt": np.ascontiguousarray((Wq[cols] / 8.0).T).astype(NP_BF16),
            "bq": np.ascontiguousarray((bq[cols] / 8.0).reshape(CC, 128).T),
            "wkt": np.ascontiguousarray(Wk[cols].T).astype(NP_BF16),
            "wvt": np.ascontiguousarray(Wv[cols].T).astype(NP_BF16),
            "bv": bv[cols].reshape(1, COLS).astype(np.float32).copy(),
            "wot": np.ascontiguousarray(Wo[:, cols].T).astype(NP_BF16),
            "bo2": np.ascontiguousarray((bo / 2.0).reshape(DCH, 128).T),
        })
    return maps


def kernel(x, Wq, bq, Wk, bk, Wv, bv, Wo, bo, _trace=False, **trace_kwargs):
    x = np.asarray(x, dtype=np.float32)
    Wq = np.asarray(Wq, dtype=np.float32)
    bq = np.asarray(bq, dtype=np.float32)
    Wk = np.asarray(Wk, dtype=np.float32)
    Wv = np.asarray(Wv, dtype=np.float32)
    bv = np.asarray(bv, dtype=np.float32)
    Wo = np.asarray(Wo, dtype=np.float32)
    bo = np.asarray(bo, dtype=np.float32)

    nc = _get_nc()
    maps = _in_maps(x, Wq, bq, Wk, Wv, bv, Wo, bo)
    res = run_bass_kernel_spmd(nc, maps, list(range(8)), trace=_trace, **trace_kwargs)

    outp = np.empty((B, S, D), np.float32)
    for b in range(B):
        t = (np.asarray(res.results[2 * b]["out"], np.float32)
             + np.asarray(res.results[2 * b + 1]["out"], np.float32))
        outp[b] = t.T
    if _trace:
        return outp, res
    return outp
